# revision 1
# baseline (speedup 1.0000x reference)
"""Trainium2 Bass kernel for ContextHyperLinearSSM.

Computes out[b,:] = x[b,:] @ (WA[context[b]] * adj_xx) + u[b,:] @ (WB[context[b]] * adj_xu)

Strategy: shard the CONTEXT axis across the 8 cores (64 contexts each).
The host groups samples by context (padded to the max group size G), so each
core streams its 64 contexts' weight banks from HBM exactly once, applies the
adjacency masks on-device, and runs 3 accumulating matmuls per context.
Each sample's row is computed by exactly one core, so the host-side unshard
is a pure scatter.

Device-side layout: contexts are processed in groups of CT; each half-group's
payload (B-weights, A-weights, x/u activations) is packed by the host into a
single contiguous HBM blob so one DMA per half-group runs at full descriptor
efficiency.  A single in-place DVE multiply against a combined [adjB|adjA]
mask tile masks a half-group's weights.  All CT contexts of a group accumulate
into one PSUM bank (two 64-aligned partition slots x two free halves), so one
ACT copy per bank drains PSUM.
"""

import sys

sys.path.insert(0, "/opt/trn_rl_repo")

import numpy as np

import concourse.bass as bass
import concourse.mybir as mybir
import concourse.tile as tile
from concourse import bacc
from concourse.bass_utils import run_bass_kernel_spmd

N_CORES = 8
CT = 8  # contexts per PSUM group
W_BUFS = 4

# matmul operand dtype: float32 (exact) or float32r (tf32-like, 4x PE rate)
MM_DT = mybir.dt.float32


def _install_profile_shim():
    """Register the NTFF profile hook that trn_boot skips when
    antenv.axon_hooks is missing from the image (profiling only)."""
    import types
    if "antenv.axon_hooks" in sys.modules:
        return
    try:
        from trn_agent_boot.trn_boot import _ntff_profile_via_ctypes
        hook = _ntff_profile_via_ctypes("/opt/axon/libaxon_pjrt.so")
    except Exception:
        hook = None
    mod = types.ModuleType("antenv.axon_hooks")
    mod.get_axon_ntff_profile_hook = lambda: hook
    mod.set_axon_ntff_profile_hook = lambda h: None
    sys.modules["antenv.axon_hooks"] = mod


def _build_program(CP, S, A, G):
    """Build the per-core Bass program. CP contexts/core, group size G."""
    f32 = mybir.dt.float32
    nc = bacc.Bacc("TRN2", target_bir_lowering=False)

    HS = S // 128  # 128-row K-chunks of the A contraction
    K = HS + 1     # matmuls per context (1 B-term + HS A-terms)
    assert S % 128 == 0 and A == 128
    NG = CP // CT
    CH = CT // 2   # contexts per half-group payload
    assert CP % CT == 0 and CT % 2 == 0
    WF = CH * K * S   # weight f32 per partition line per half-group
    AF = CH * K * G   # activation f32 per partition line per half-group

    # PSUM packing: FF contexts along the free dim of a bank, two 64-aligned
    # partition slots (matmul out base partition must be 0/32/64)
    FF = max(1, min(CT, 512 // S))
    PSL = 2 if G <= 64 else 1
    CPT = min(CT, PSL * FF)
    T = -(-CT // CPT)
    assert T * CPT == CT, (CT, FF, PSL, CPT)

    blob = nc.dram_tensor("blob", [NG, 2, 128, WF + AF], f32,
                          kind="ExternalInput").ap()
    adj_xx = nc.dram_tensor("adj_xx", [HS, 128, S], mybir.dt.uint8,
                            kind="ExternalInput").ap()
    adj_xu = nc.dram_tensor("adj_xu", [A, S], mybir.dt.uint8,
                            kind="ExternalInput").ap()
    # output blob: [group][partition-slot][sample][bank][context-half][s]
    out = nc.dram_tensor("out", [NG, PSL, G, T, FF, S], f32,
                         kind="ExternalOutput").ap()

    rounded = MM_DT == mybir.dt.float32r

    with tile.TileContext(nc) as tc:
        with (
            tc.tile_pool(name="const", bufs=1) as const,
            tc.tile_pool(name="w", bufs=W_BUFS) as wpool,
            tc.tile_pool(name="o", bufs=3) as opool,
            tc.tile_pool(name="psum", bufs=8, space="PSUM") as psum,
        ):
            # combined [adjB | adjA] mask: raw u8 over the fast HWDGE ring,
            # then one DVE cast-copy (also the same-engine funnel for the
            # mask TTs).  SWDGE cast-DMAs would cost ~17us of ramp-in.
            adjU = const.tile([128, K * S], mybir.dt.uint8)
            nc.sync.dma_start(adjU[:, :S], adj_xu[:])
            nc.sync.dma_start(
                adjU[:, S:].rearrange("p (h s) -> p h s", h=HS),
                adj_xx.rearrange("h p s -> p h s"))
            adjC = const.tile([128, K * S], f32)
            nc.vector.tensor_copy(adjC[:], adjU[:])
            adjC_b = adjC[:, None, :].to_broadcast([128, CH, K * S])

            for g in range(NG):
                halves = []
                for hf in range(2):
                    hb = wpool.tile([128, WF + AF], f32, tag="hb",
                                    name=f"hb_{g}_{hf}")
                    nc.sync.dma_start(hb[:], blob[g, hf])
                    wv = hb[:, :WF].rearrange("p (c k s) -> p c k s",
                                              c=CH, k=K)
                    av = hb[:, WF:].rearrange("p (c k g) -> p c k g",
                                              c=CH, k=K)
                    if rounded:
                        wm = wpool.tile([128, WF], MM_DT, tag="wm",
                                        name=f"wm_{g}_{hf}")
                        am = wpool.tile([128, AF], MM_DT, tag="am",
                                        name=f"am_{g}_{hf}")
                        nc.vector.tensor_copy(am[:], hb[:, WF:])
                        nc.vector.tensor_tensor(
                            wm[:].rearrange("p (c ks) -> p c ks", c=CH),
                            hb[:, :WF].rearrange("p (c ks) -> p c ks", c=CH),
                            adjC_b, mybir.AluOpType.mult)
                        wv = wm[:].rearrange("p (c k s) -> p c k s",
                                             c=CH, k=K)
                        av = am[:].rearrange("p (c k g) -> p c k g",
                                             c=CH, k=K)
                    else:
                        # mask B+A weights with ONE in-place multiply
                        nc.vector.tensor_tensor(
                            hb[:, :WF].rearrange("p (c ks) -> p c ks", c=CH),
                            hb[:, :WF].rearrange("p (c ks) -> p c ks", c=CH),
                            adjC_b, mybir.AluOpType.mult)
                    halves.append((wv, av))

                ps_tiles = [psum.tile([128, FF * S], f32, tag="ps",
                                      name=f"ps_{g}_{t}")
                            for t in range(T)]
                for c in range(CT):
                    hf, ci = divmod(c, CH)
                    wv, av = halves[hf]
                    t, r2 = divmod(c, CPT)
                    sl, cf = divmod(r2, FF)
                    pslice = ps_tiles[t][sl * 64:sl * 64 + G,
                                         cf * S:cf * S + S]
                    for k in range(K):
                        nc.tensor.matmul(
                            pslice,
                            lhsT=av[:, ci, k, :],
                            rhs=wv[:, ci, k, :],
                            start=(k == 0), stop=(k == K - 1))
                out_sb = opool.tile([128, T, FF, S], f32)
                for t in range(T):
                    for sl in range(PSL):
                        nc.scalar.copy(
                            out_sb[sl * 64:sl * 64 + G, t].rearrange(
                                "p f s -> p (f s)"),
                            ps_tiles[t][sl * 64:sl * 64 + G, :])
                for sl in range(PSL):
                    nc.scalar.dma_start(
                        out[g, sl], out_sb[sl * 64:sl * 64 + G])

    nc.compile()
    return nc


def kernel(x, u, WA, WB, adj_xx, adj_xu, context, _trace=False):
    B, S = x.shape
    _, A = u.shape
    C = WA.shape[0]
    assert C % N_CORES == 0
    CP = C // N_CORES
    HS = S // 128
    K = HS + 1
    NG = CP // CT
    CH = CT // 2

    # ---- host-side shard: group samples by context --------------------
    context = np.asarray(context)
    cnt = np.bincount(context, minlength=C)
    G = int(cnt.max())
    G = max(4, ((G + 3) // 4) * 4)
    order = np.argsort(context, kind="stable")
    starts = np.zeros(C + 1, np.int64)
    starts[1:] = np.cumsum(cnt)
    j = np.arange(G)
    valid = j[None, :] < cnt[:, None]                      # [C, G]
    pos = starts[:-1, None] + np.minimum(j[None, :],
                                         np.maximum(cnt[:, None] - 1, 0))
    gidx = order[pos]                                      # [C, G]

    Xp = np.asarray(x, np.float32)[gidx]                   # [C, G, S]
    Up = np.asarray(u, np.float32)[gidx]                   # [C, G, A]
    XpT = np.ascontiguousarray(Xp.transpose(0, 2, 1))      # [C, S, G]
    UpT = np.ascontiguousarray(Up.transpose(0, 2, 1))      # [C, A, G]

    WA = np.ascontiguousarray(WA, np.float32)
    WB = np.ascontiguousarray(WB, np.float32)
    adjxx_u8 = np.ascontiguousarray(adj_xx).view(np.uint8).reshape(HS, 128, S)
    adjxu_u8 = np.ascontiguousarray(adj_xu).view(np.uint8)

    WF = CH * K * S
    AF = CH * K * G
    in_maps = []
    for k in range(N_CORES):
        sl = slice(k * CP, (k + 1) * CP)
        # pack each half-group's weights+activations into one contiguous
        # blob: per partition line [CH, K, S] weights then [CH, K, G] acts,
        # slot 0 = B-term, slot 1+h = A-term K-chunk h
        W3 = np.empty((NG, 2, 128, CH, K, S), np.float32)
        W3[..., 0, :] = WB[sl].reshape(NG, 2, CH, 128, S) \
            .transpose(0, 1, 3, 2, 4)
        W3[..., 1:, :] = WA[sl].reshape(NG, 2, CH, HS, 128, S) \
            .transpose(0, 1, 4, 2, 3, 5)
        A3 = np.empty((NG, 2, 128, CH, K, G), np.float32)
        A3[..., 0, :] = UpT[sl].reshape(NG, 2, CH, 128, G) \
            .transpose(0, 1, 3, 2, 4)
        A3[..., 1:, :] = XpT[sl].reshape(NG, 2, CH, HS, 128, G) \
            .transpose(0, 1, 4, 2, 3, 5)
        blob = np.concatenate(
            [W3.reshape(NG, 2, 128, WF), A3.reshape(NG, 2, 128, AF)],
            axis=-1)
        in_maps.append({
            "blob": np.ascontiguousarray(blob),
            "adj_xx": adjxx_u8,
            "adj_xu": adjxu_u8,
        })

    if _trace:
        _install_profile_shim()
    nc = _build_program(CP, S, A, G)
    res = run_bass_kernel_spmd(nc, in_maps, core_ids=list(range(N_CORES)),
                               trace=_trace)

    # device output blobs [NG, PSL, G, T, FF, S] -> [CP, G, S].
    # context c in a group lives at bank t=c//CPT, partition slot
    # sl=(c%CPT)//FF (64-aligned), free half cf=c%FF.
    outs = []
    for r in res.results:
        v = r["out"]
        # axes (g, sl, gg, t, cf, s) -> (g, t, sl, cf, gg, s)
        v = v.transpose(0, 3, 1, 4, 2, 5).reshape(CP, G, S)
        outs.append(v)
    Out_all = np.concatenate(outs, axis=0)                 # [C, G, S]
    out_full = np.zeros((B, S), np.float32)
    out_full[gidx[valid]] = Out_all[valid]

    if _trace:
        return out_full, res
    return out_full



# revision 2
# speedup vs baseline: 1.6893x; 1.6893x over previous
"""Trainium2 Bass kernel for ContextHyperLinearSSM.

Computes out[b,:] = x[b,:] @ (WA[context[b]] * adj_xx) + u[b,:] @ (WB[context[b]] * adj_xu)

Strategy: shard the CONTEXT axis across the 8 cores (64 contexts each).
The host groups samples by context (padded to the max group size G), masks
the weight banks with the adjacency masks, and converts weights+activations
to bf16.  Each core streams its 64 contexts' pre-masked bf16 weights from
HBM exactly once and runs 3 accumulating matmuls per context.  Each
sample's row is computed by exactly one core, so the host-side unshard is a
pure scatter.

Device-side layout: contexts are processed in groups of CT; each half-group's
payload (B-weights, A-weights, x/u activations) is packed by the host into a
single contiguous HBM blob so one DMA per half-group runs at full descriptor
efficiency.  All CT contexts of a group accumulate into T PSUM banks (two
64-aligned partition slots x two free halves); one full-width DVE copy per
bank drains PSUM to a bf16 staging tile that DMAs out.
"""

import sys

sys.path.insert(0, "/opt/trn_rl_repo")

import ml_dtypes
import numpy as np

import concourse.bass as bass
import concourse.mybir as mybir
import concourse.tile as tile
from concourse import bacc
from concourse.bass_utils import run_bass_kernel_spmd

N_CORES = 8
CT = 8  # contexts per PSUM group
W_BUFS = 4

BF16 = ml_dtypes.bfloat16


def _install_profile_shim():
    """Register the NTFF profile hook that trn_boot skips when
    antenv.axon_hooks is missing from the image (profiling only)."""
    import types
    if "antenv.axon_hooks" in sys.modules:
        return
    try:
        from trn_agent_boot.trn_boot import _ntff_profile_via_ctypes
        hook = _ntff_profile_via_ctypes("/opt/axon/libaxon_pjrt.so")
    except Exception:
        hook = None
    mod = types.ModuleType("antenv.axon_hooks")
    mod.get_axon_ntff_profile_hook = lambda: hook
    mod.set_axon_ntff_profile_hook = lambda h: None
    sys.modules["antenv.axon_hooks"] = mod


def _build_program(CP, S, A, G):
    """Build the per-core Bass program. CP contexts/core, group size G."""
    f32 = mybir.dt.float32
    bf16 = mybir.dt.bfloat16
    nc = bacc.Bacc("TRN2", target_bir_lowering=False)

    HS = S // 128  # 128-row K-chunks of the A contraction
    K = HS + 1     # matmuls per context (1 B-term + HS A-terms)
    assert S % 128 == 0 and A == 128
    NG = CP // CT
    CH = CT // 2   # contexts per half-group payload
    assert CP % CT == 0 and CT % 2 == 0
    WF = CH * K * S   # weight elems per partition line per half-group
    AF = CH * K * G   # activation elems per partition line per half-group

    # PSUM packing: FF contexts along the free dim of a bank, two 64-aligned
    # partition slots (matmul out base partition must be 0/32/64)
    FF = max(1, min(CT, 512 // S))
    PSL = 2 if G <= 64 else 1
    CPT = min(CT, PSL * FF)
    T = -(-CT // CPT)
    assert T * CPT == CT, (CT, FF, PSL, CPT)

    blob = nc.dram_tensor("blob", [NG, 2, 128, WF + AF], bf16,
                          kind="ExternalInput").ap()
    # output blob: [group][partition-slot][sample][bank][context-half][s]
    out = nc.dram_tensor("out", [NG, PSL, G, T, FF, S], bf16,
                         kind="ExternalOutput").ap()

    with tile.TileContext(nc) as tc:
        with (
            tc.tile_pool(name="w", bufs=W_BUFS) as wpool,
            tc.tile_pool(name="o", bufs=3) as opool,
            tc.tile_pool(name="psum", bufs=8, space="PSUM") as psum,
        ):
            for g in range(NG):
                halves = []
                for hf in range(2):
                    hb = wpool.tile([128, WF + AF], bf16, tag="hb",
                                    name=f"hb_{g}_{hf}")
                    nc.sync.dma_start(hb[:], blob[g, hf])
                    wv = hb[:, :WF].rearrange("p (c k s) -> p c k s",
                                              c=CH, k=K)
                    av = hb[:, WF:].rearrange("p (c k g) -> p c k g",
                                              c=CH, k=K)
                    halves.append((wv, av))

                ps_tiles = [psum.tile([128, FF * S], f32, tag="ps",
                                      name=f"ps_{g}_{t}")
                            for t in range(T)]
                for c in range(CT):
                    hf, ci = divmod(c, CH)
                    wv, av = halves[hf]
                    t, r2 = divmod(c, CPT)
                    sl, cf = divmod(r2, FF)
                    pslice = ps_tiles[t][sl * 64:sl * 64 + G,
                                         cf * S:cf * S + S]
                    for k in range(K):
                        nc.tensor.matmul(
                            pslice,
                            lhsT=av[:, ci, k, :],
                            rhs=wv[:, ci, k, :],
                            start=(k == 0), stop=(k == K - 1))
                out_sb = opool.tile([128, T, FF, S], bf16)
                for t in range(T):
                    # one full-width drain per bank (garbage rows are
                    # never DMA'd out)
                    nc.vector.tensor_copy(
                        out_sb[:, t].rearrange("p f s -> p (f s)"),
                        ps_tiles[t][:, :])
                for sl in range(PSL):
                    nc.scalar.dma_start(
                        out[g, sl], out_sb[sl * 64:sl * 64 + G])

    nc.compile()
    return nc


def kernel(x, u, WA, WB, adj_xx, adj_xu, context, _trace=False):
    B, S = x.shape
    _, A = u.shape
    C = WA.shape[0]
    assert C % N_CORES == 0
    CP = C // N_CORES
    HS = S // 128
    K = HS + 1
    NG = CP // CT
    CH = CT // 2

    # ---- host-side shard: group samples by context --------------------
    context = np.asarray(context)
    cnt = np.bincount(context, minlength=C)
    G = int(cnt.max())
    G = max(4, ((G + 3) // 4) * 4)
    order = np.argsort(context, kind="stable")
    starts = np.zeros(C + 1, np.int64)
    starts[1:] = np.cumsum(cnt)
    j = np.arange(G)
    valid = j[None, :] < cnt[:, None]                      # [C, G]
    pos = starts[:-1, None] + np.minimum(j[None, :],
                                         np.maximum(cnt[:, None] - 1, 0))
    gidx = order[pos]                                      # [C, G]

    Xp = np.asarray(x, np.float32)[gidx]                   # [C, G, S]
    Up = np.asarray(u, np.float32)[gidx]                   # [C, G, A]
    XpT = np.ascontiguousarray(Xp.transpose(0, 2, 1)).astype(BF16)
    UpT = np.ascontiguousarray(Up.transpose(0, 2, 1)).astype(BF16)

    # pre-mask the weight banks and quantize to bf16 on the host
    Am = (np.asarray(WA, np.float32)
          * np.asarray(adj_xx, np.float32)).astype(BF16)   # [C, S, S]
    Bm = (np.asarray(WB, np.float32)
          * np.asarray(adj_xu, np.float32)).astype(BF16)   # [C, A, S]

    WF = CH * K * S
    AF = CH * K * G
    in_maps = []
    for k in range(N_CORES):
        sl = slice(k * CP, (k + 1) * CP)
        # pack each half-group's weights+activations into one contiguous
        # blob: per partition line [CH, K, S] weights then [CH, K, G] acts,
        # slot 0 = B-term, slot 1+h = A-term K-chunk h
        W3 = np.empty((NG, 2, 128, CH, K, S), BF16)
        W3[..., 0, :] = Bm[sl].reshape(NG, 2, CH, 128, S) \
            .transpose(0, 1, 3, 2, 4)
        W3[..., 1:, :] = Am[sl].reshape(NG, 2, CH, HS, 128, S) \
            .transpose(0, 1, 4, 2, 3, 5)
        A3 = np.empty((NG, 2, 128, CH, K, G), BF16)
        A3[..., 0, :] = UpT[sl].reshape(NG, 2, CH, 128, G) \
            .transpose(0, 1, 3, 2, 4)
        A3[..., 1:, :] = XpT[sl].reshape(NG, 2, CH, HS, 128, G) \
            .transpose(0, 1, 4, 2, 3, 5)
        blob = np.concatenate(
            [W3.reshape(NG, 2, 128, WF), A3.reshape(NG, 2, 128, AF)],
            axis=-1)
        in_maps.append({"blob": np.ascontiguousarray(blob)})

    if _trace:
        _install_profile_shim()
    nc = _build_program(CP, S, A, G)
    res = run_bass_kernel_spmd(nc, in_maps, core_ids=list(range(N_CORES)),
                               trace=_trace)

    # device output blobs [NG, PSL, G, T, FF, S] -> [CP, G, S].
    # context c in a group lives at bank t=c//CPT, partition slot
    # sl=(c%CPT)//FF (64-aligned), free half cf=c%FF.
    outs = []
    for r in res.results:
        v = np.asarray(r["out"]).astype(np.float32)
        # axes (g, sl, gg, t, cf, s) -> (g, t, sl, cf, gg, s)
        v = v.transpose(0, 3, 1, 4, 2, 5).reshape(CP, G, S)
        outs.append(v)
    Out_all = np.concatenate(outs, axis=0)                 # [C, G, S]
    out_full = np.zeros((B, S), np.float32)
    out_full[gidx[valid]] = Out_all[valid]

    if _trace:
        return out_full, res

    return out_full


# revision 3
# speedup vs baseline: 1.7757x; 1.0512x over previous
"""Trainium2 Bass kernel for ContextHyperLinearSSM.

Computes out[b,:] = x[b,:] @ (WA[context[b]] * adj_xx) + u[b,:] @ (WB[context[b]] * adj_xu)

Strategy: shard the CONTEXT axis across the 8 cores (64 contexts each).
The host groups samples by context, masks the weight banks with the
adjacency masks, and converts weights+activations to bf16.  Each core
streams its 64 contexts' pre-masked bf16 weights from HBM exactly once and
runs 3 accumulating matmuls per context.  Each sample's row is computed by
exactly one core, so the host-side unshard is a pure scatter.

Contexts are globally sorted by sample count and dealt round-robin to the
cores, so every core sees the same per-group padded size G_g (required:
one SPMD program serves all cores) and the padding tracks the count
distribution instead of the global max.  Groups are processed largest
first, so the tail group is the smallest; its blob is additionally split
into per-context DMAs so the final matmuls start as soon as their slice of
the last transfer lands.

Device-side layout: contexts are processed in groups of CT; each half-
group's payload (B-weights, A-weights, x/u activations) is packed by the
host into one contiguous HBM blob so one DMA per half-group runs at full
descriptor efficiency.  All CT contexts of a group accumulate into T PSUM
banks (two 64-aligned partition slots x two free halves); one full-width
copy per bank (vector engine for bank 0, scalar for bank 1) drains PSUM to
a bf16 staging tile that DMAs out.
"""

import sys

sys.path.insert(0, "/opt/trn_rl_repo")

import ml_dtypes
import numpy as np

import concourse.bass as bass
import concourse.mybir as mybir
import concourse.tile as tile
from concourse import bacc
from concourse.bass_utils import run_bass_kernel_spmd

N_CORES = 8
CT = 8  # contexts per PSUM group
W_BUFS = 4

BF16 = ml_dtypes.bfloat16


def _install_profile_shim():
    """Register the NTFF profile hook that trn_boot skips when
    antenv.axon_hooks is missing from the image (profiling only)."""
    import types
    if "antenv.axon_hooks" in sys.modules:
        return
    try:
        from trn_agent_boot.trn_boot import _ntff_profile_via_ctypes
        hook = _ntff_profile_via_ctypes("/opt/axon/libaxon_pjrt.so")
    except Exception:
        hook = None
    mod = types.ModuleType("antenv.axon_hooks")
    mod.get_axon_ntff_profile_hook = lambda: hook
    mod.set_axon_ntff_profile_hook = lambda h: None
    sys.modules["antenv.axon_hooks"] = mod


def _group_geometry(S, A, Gs):
    """Static per-group blob/psum geometry shared by host and device."""
    HS = S // 128
    K = HS + 1
    CH = CT // 2
    WF = CH * K * S
    geo = []
    off_in = 0
    off_out = 0
    for G in Gs:
        AF = CH * K * G
        FF = max(1, min(CT, 512 // S))
        PSL = 2 if G <= 64 else 1
        CPT = min(CT, PSL * FF)
        T = -(-CT // CPT)
        assert T * CPT == CT
        geo.append(dict(G=G, AF=AF, FF=FF, PSL=PSL, CPT=CPT, T=T,
                        off_in=off_in, off_out=off_out))
        off_in += 2 * 128 * (WF + AF)
        off_out += PSL * G * T * FF * S
    return geo, off_in, off_out, WF, K, HS, CH


def _build_program(S, A, Gs):
    """Build the per-core Bass program for per-group sizes Gs."""
    f32 = mybir.dt.float32
    bf16 = mybir.dt.bfloat16
    nc = bacc.Bacc("TRN2", target_bir_lowering=False)

    geo, tot_in, tot_out, WF, K, HS, CH = _group_geometry(S, A, Gs)
    NG = len(Gs)
    assert S % 128 == 0 and A == 128

    blob = nc.dram_tensor("blob", [tot_in], bf16, kind="ExternalInput").ap()
    out = nc.dram_tensor("out", [tot_out], bf16, kind="ExternalOutput").ap()

    with tile.TileContext(nc) as tc:
        with (
            tc.tile_pool(name="w", bufs=W_BUFS) as wpool,
            tc.tile_pool(name="o", bufs=3) as opool,
            tc.tile_pool(name="psum", bufs=8, space="PSUM") as psum,
        ):
            for g, gg in enumerate(geo):
                G, AF, FF, PSL, CPT, T = (gg["G"], gg["AF"], gg["FF"],
                                          gg["PSL"], gg["CPT"], gg["T"])
                L = WF + AF
                halves = []
                for hf in range(2):
                    hb = wpool.tile([128, L], bf16, tag="hb",
                                    name=f"hb_{g}_{hf}")
                    src = blob[gg["off_in"] + hf * 128 * L:
                               gg["off_in"] + (hf + 1) * 128 * L]
                    src = src.rearrange("(p l) -> p l", p=128)
                    if g == NG - 1:
                        # tail group: per-context weight DMAs so the last
                        # matmuls start as soon as their slice lands
                        nc.sync.dma_start(hb[:, WF:], src[:, WF:])
                        for ci in range(CH):
                            nc.sync.dma_start(
                                hb[:, ci * K * S:(ci + 1) * K * S],
                                src[:, ci * K * S:(ci + 1) * K * S])
                    else:
                        nc.sync.dma_start(hb[:], src)
                    wv = hb[:, :WF].rearrange("p (c k s) -> p c k s",
                                              c=CH, k=K)
                    av = hb[:, WF:].rearrange("p (c k g) -> p c k g",
                                              c=CH, k=K)
                    halves.append((wv, av))

                ps_tiles = [psum.tile([128, FF * S], f32, tag="ps",
                                      name=f"ps_{g}_{t}")
                            for t in range(T)]
                for c in range(CT):
                    hf, ci = divmod(c, CH)
                    wv, av = halves[hf]
                    t, r2 = divmod(c, CPT)
                    sl, cf = divmod(r2, FF)
                    pslice = ps_tiles[t][sl * 64:sl * 64 + G,
                                         cf * S:cf * S + S]
                    for k in range(K):
                        nc.tensor.matmul(
                            pslice,
                            lhsT=av[:, ci, k, :],
                            rhs=wv[:, ci, k, :],
                            start=(k == 0), stop=(k == K - 1))
                out_sb = opool.tile([128, T, FF, S], bf16)
                for t in range(T):
                    # one full-width drain per bank (garbage rows are never
                    # DMA'd out); banks split across vector+scalar engines
                    eng = nc.vector.tensor_copy if t == 0 else nc.scalar.copy
                    eng(out_sb[:, t].rearrange("p f s -> p (f s)"),
                        ps_tiles[t][:, :])
                for sl in range(PSL):
                    dst = out[gg["off_out"] + sl * G * T * FF * S:
                              gg["off_out"] + (sl + 1) * G * T * FF * S]
                    nc.scalar.dma_start(
                        dst.rearrange("(gg l) -> gg l", gg=G),
                        out_sb[sl * 64:sl * 64 + G].rearrange(
                            "p t f s -> p (t f s)"))

    nc.compile()
    return nc


def kernel(x, u, WA, WB, adj_xx, adj_xu, context, _trace=False):
    B, S = x.shape
    _, A = u.shape
    C = WA.shape[0]
    assert C % N_CORES == 0
    CP = C // N_CORES
    assert CP % CT == 0
    NG = CP // CT
    HS = S // 128
    K = HS + 1
    CH = CT // 2

    # ---- host-side shard: count-sorted contexts, dealt round-robin ----
    context = np.asarray(context)
    cnt = np.bincount(context, minlength=C)
    perm = np.argsort(-cnt, kind="stable")          # contexts by count desc
    # context at global rank r -> core r%8, position r//8; group = pos//CT.
    # All cores share one program, so G_g is set by the chunk's global max
    # count = count at rank g*CT*N_CORES.
    Gs = []
    for g in range(NG):
        m = int(cnt[perm[g * CT * N_CORES]])
        Gs.append(max(2, ((m + 1) // 2) * 2))

    order = np.argsort(context, kind="stable")
    starts = np.zeros(C + 1, np.int64)
    starts[1:] = np.cumsum(cnt)

    def group_rows(ctx_ids, G):
        """gidx [len,G] sample indices (clamped) + valid mask."""
        j = np.arange(G)
        cc = cnt[ctx_ids][:, None]
        valid = j[None, :] < cc
        pos = starts[ctx_ids][:, None] + np.minimum(j[None, :],
                                                    np.maximum(cc - 1, 0))
        return order[pos], valid

    x = np.asarray(x, np.float32)
    u = np.asarray(u, np.float32)

    # pre-mask the weight banks and quantize to bf16 on the host
    Am = (np.asarray(WA, np.float32)
          * np.asarray(adj_xx, np.float32)).astype(BF16)   # [C, S, S]
    Bm = (np.asarray(WB, np.float32)
          * np.asarray(adj_xu, np.float32)).astype(BF16)   # [C, A, S]

    geo, tot_in, tot_out, WF, _, _, _ = _group_geometry(S, A, Gs)

    in_maps = []
    scatter = []   # per core: list of (gidx, valid) per group
    for k in range(N_CORES):
        blob = np.zeros(tot_in, BF16)
        sc = []
        for g, gg in enumerate(geo):
            G = gg["G"]
            ctx_ids = perm[(g * CT + np.arange(CT)) * N_CORES + k]
            gidx, valid = group_rows(ctx_ids, G)           # [CT, G]
            sc.append((ctx_ids, gidx, valid))
            Xp = x[gidx]                                   # [CT, G, S]
            Up = u[gidx]                                   # [CT, G, A]
            XpT = Xp.transpose(0, 2, 1).astype(BF16)       # [CT, S, G]
            UpT = Up.transpose(0, 2, 1).astype(BF16)       # [CT, A, G]
            L = WF + gg["AF"]
            gb = blob[gg["off_in"]: gg["off_in"] + 2 * 128 * L] \
                .reshape(2, 128, L)
            W3 = gb[:, :, :WF].reshape(2, 128, CH, K, S)
            A3 = gb[:, :, WF:].reshape(2, 128, CH, K, G)
            W3[..., 0, :] = Bm[ctx_ids].reshape(2, CH, 128, S) \
                .transpose(0, 2, 1, 3)
            W3[..., 1:, :] = Am[ctx_ids].reshape(2, CH, HS, 128, S) \
                .transpose(0, 3, 1, 2, 4)
            A3[..., 0, :] = UpT.reshape(2, CH, 128, G).transpose(0, 2, 1, 3)
            A3[..., 1:, :] = XpT.reshape(2, CH, HS, 128, G) \
                .transpose(0, 3, 1, 2, 4)
        in_maps.append({"blob": blob})
        scatter.append(sc)

    if _trace:
        _install_profile_shim()
    nc = _build_program(S, A, Gs)
    res = run_bass_kernel_spmd(nc, in_maps, core_ids=list(range(N_CORES)),
                               trace=_trace)

    # device output blob per group: [PSL, G, T, FF, S]; context c of the
    # group lives at bank t=c//CPT, partition slot sl=(c%CPT)//FF, free
    # half cf=c%FF -> row block [sl, :, t, cf, :].
    out_full = np.zeros((B, S), np.float32)
    for k, r in enumerate(res.results):
        v = np.asarray(r["out"]).astype(np.float32)
        for g, gg in enumerate(geo):
            G, T, FF, PSL, CPT = (gg["G"], gg["T"], gg["FF"], gg["PSL"],
                                  gg["CPT"])
            blk = v[gg["off_out"]: gg["off_out"] + PSL * G * T * FF * S] \
                .reshape(PSL, G, T, FF, S)
            ctx_ids, gidx, valid = scatter[k][g]
            for c in range(CT):
                t, r2 = divmod(c, CPT)
                sl, cf = divmod(r2, FF)
                rows = blk[sl, :, t, cf, :]                # [G, S]
                m = valid[c]
                out_full[gidx[c][m]] = rows[m]

    if _trace:
        return out_full, res

    return out_full


# revision 5
# speedup vs baseline: 2.2146x; 1.2471x over previous
"""Trainium2 Bass kernel for ContextHyperLinearSSM.

Computes out[b,:] = x[b,:] @ (WA[context[b]] * adj_xx) + u[b,:] @ (WB[context[b]] * adj_xu)

Strategy: shard the CONTEXT axis across the 8 cores (64 contexts each).
The host groups samples by context, masks the weight banks with the
adjacency masks, and quantizes the weights to fp8-e3m4 (x2^6 scale; the
inverse 2^-6 is folded into the bf16 activations — both scalings are exact
powers of two, so the only quantization error is the e3m4 weight rounding,
measured at 1.3e-2 absmax/scale against the fp32 reference).  Each core
streams its 64 contexts' weights from HBM exactly once and runs 3
accumulating mixed-dtype matmuls (bf16 stationary x fp8 moving) per
context.  Each sample's row is computed by exactly one core, so the
host-side unshard is a pure scatter.

Contexts are globally sorted by sample count and dealt round-robin to the
cores, so every core sees the same per-group padded size G_g (required:
one SPMD program serves all cores) and the padding tracks the count
distribution instead of the global max.  All activations ship in one
prefetched DMA; weights stream per half-group.  Groups are processed
largest first, so the tail group is the smallest; its weights are
additionally split into per-context DMAs so the final matmuls start as
soon as their slice of the last transfer lands.

All CT contexts of a group accumulate into T PSUM banks (two 64-aligned
partition slots x two free halves); one full-width copy per bank (vector
engine for bank 0, scalar for bank 1) drains PSUM to a bf16 staging tile
that DMAs out.
"""

import sys

sys.path.insert(0, "/opt/trn_rl_repo")

import ml_dtypes
import numpy as np

import concourse.bass as bass
import concourse.mybir as mybir
import concourse.tile as tile
from concourse import bacc
from concourse.bass_utils import run_bass_kernel_spmd

N_CORES = 8
CT = 8  # contexts per PSUM group
W_BUFS = 4
WSCALE = 64.0  # 2^6: weights *= WSCALE (into e3m4 range), acts /= WSCALE

BF16 = ml_dtypes.bfloat16
FP8 = ml_dtypes.float8_e3m4


def _install_profile_shim():
    """Register the NTFF profile hook that trn_boot skips when
    antenv.axon_hooks is missing from the image (profiling only)."""
    import types
    if "antenv.axon_hooks" in sys.modules:
        return
    try:
        from trn_agent_boot.trn_boot import _ntff_profile_via_ctypes
        hook = _ntff_profile_via_ctypes("/opt/axon/libaxon_pjrt.so")
    except Exception:
        hook = None
    mod = types.ModuleType("antenv.axon_hooks")
    mod.get_axon_ntff_profile_hook = lambda: hook
    mod.set_axon_ntff_profile_hook = lambda h: None
    sys.modules["antenv.axon_hooks"] = mod


def _group_geometry(S, A, Gs):
    """Static per-group weight/act/psum geometry shared by host and device."""
    HS = S // 128
    K = HS + 1
    CH = CT // 2
    WF = CH * K * S          # weight elems per partition line per half-group
    geo = []
    off_a = 0                # acts offset, in elems per partition line
    off_out = 0
    for G in Gs:
        AF = CH * K * G
        FF = max(1, min(CT, 512 // S))
        PSL = 2 if G <= 64 else 1
        CPT = min(CT, PSL * FF)
        T = -(-CT // CPT)
        assert T * CPT == CT
        geo.append(dict(G=G, AF=AF, FF=FF, PSL=PSL, CPT=CPT, T=T,
                        off_a=off_a, off_out=off_out))
        off_a += 2 * AF
        off_out += PSL * G * T * FF * S
    return geo, off_a, off_out, WF, K, HS, CH


def _build_program(S, A, Gs):
    """Build the per-core Bass program for per-group sizes Gs."""
    f32 = mybir.dt.float32
    bf16 = mybir.dt.bfloat16
    fp8 = mybir.dt.float8e3
    nc = bacc.Bacc("TRN2", target_bir_lowering=False)

    geo, AL, tot_out, WF, K, HS, CH = _group_geometry(S, A, Gs)
    NG = len(Gs)
    assert S % 128 == 0 and A == 128

    wts = nc.dram_tensor("wts", [NG, 2, 128, WF], fp8,
                         kind="ExternalInput").ap()
    acts = nc.dram_tensor("acts", [128, AL], bf16,
                          kind="ExternalInput").ap()
    out = nc.dram_tensor("out", [tot_out], bf16, kind="ExternalOutput").ap()

    with tile.TileContext(nc) as tc:
        with (
            tc.tile_pool(name="a", bufs=1) as apool,
            tc.tile_pool(name="w", bufs=W_BUFS) as wpool,
            tc.tile_pool(name="o", bufs=3) as opool,
            tc.tile_pool(name="psum", bufs=8, space="PSUM") as psum,
        ):
            # all activations prefetched in one DMA (small: ~4KB/partition)
            at = apool.tile([128, AL], bf16)
            nc.sync.dma_start(at[:], acts[:])

            for g, gg in enumerate(geo):
                G, FF, PSL, CPT, T = (gg["G"], gg["FF"], gg["PSL"],
                                      gg["CPT"], gg["T"])
                halves = []
                for hf in range(2):
                    wt = wpool.tile([128, WF], fp8, tag="wt",
                                    name=f"wt_{g}_{hf}")
                    if g == NG - 1:
                        # tail group: per-context weight DMAs so the last
                        # matmuls start as soon as their slice lands
                        for ci in range(CH):
                            nc.sync.dma_start(
                                wt[:, ci * K * S:(ci + 1) * K * S],
                                wts[g, hf, :, ci * K * S:(ci + 1) * K * S])
                    else:
                        nc.sync.dma_start(wt[:], wts[g, hf])
                    wv = wt[:].rearrange("p (c k s) -> p c k s", c=CH, k=K)
                    av = at[:, gg["off_a"] + hf * CH * K * G:
                            gg["off_a"] + (hf + 1) * CH * K * G] \
                        .rearrange("p (c k g) -> p c k g", c=CH, k=K)
                    halves.append((wv, av))

                ps_tiles = [psum.tile([128, FF * S], f32, tag="ps",
                                      name=f"ps_{g}_{t}")
                            for t in range(T)]
                for c in range(CT):
                    hf, ci = divmod(c, CH)
                    wv, av = halves[hf]
                    t, r2 = divmod(c, CPT)
                    sl, cf = divmod(r2, FF)
                    pslice = ps_tiles[t][sl * 64:sl * 64 + G,
                                         cf * S:cf * S + S]
                    for k in range(K):
                        nc.tensor.matmul(
                            pslice,
                            lhsT=av[:, ci, k, :],
                            rhs=wv[:, ci, k, :],
                            start=(k == 0), stop=(k == K - 1))
                out_sb = opool.tile([128, T, FF, S], bf16)
                for t in range(T):
                    # one full-width drain per bank (garbage rows are never
                    # DMA'd out); banks split across vector+scalar engines
                    eng = nc.vector.tensor_copy if t == 0 else nc.scalar.copy
                    eng(out_sb[:, t].rearrange("p f s -> p (f s)"),
                        ps_tiles[t][:, :])
                for sl in range(PSL):
                    dst = out[gg["off_out"] + sl * G * T * FF * S:
                              gg["off_out"] + (sl + 1) * G * T * FF * S]
                    nc.scalar.dma_start(
                        dst.rearrange("(gg l) -> gg l", gg=G),
                        out_sb[sl * 64:sl * 64 + G].rearrange(
                            "p t f s -> p (t f s)"))

    nc.compile()
    return nc


def kernel(x, u, WA, WB, adj_xx, adj_xu, context, _trace=False):
    B, S = x.shape
    _, A = u.shape
    C = WA.shape[0]
    assert C % N_CORES == 0
    CP = C // N_CORES
    assert CP % CT == 0
    NG = CP // CT
    HS = S // 128
    K = HS + 1
    CH = CT // 2

    # ---- host-side shard: count-sorted contexts, dealt round-robin ----
    context = np.asarray(context)
    cnt = np.bincount(context, minlength=C)
    perm = np.argsort(-cnt, kind="stable")          # contexts by count desc
    # context at global rank r -> core r%8, position r//8; group = pos//CT.
    # All cores share one program, so G_g is set by the chunk's global max
    # count = count at rank g*CT*N_CORES.
    Gs = []
    for g in range(NG):
        m = int(cnt[perm[g * CT * N_CORES]])
        Gs.append(max(2, ((m + 1) // 2) * 2))

    order = np.argsort(context, kind="stable")
    starts = np.zeros(C + 1, np.int64)
    starts[1:] = np.cumsum(cnt)

    def group_rows(ctx_ids, G):
        """gidx [len,G] sample indices (clamped) + valid mask."""
        j = np.arange(G)
        cc = cnt[ctx_ids][:, None]
        valid = j[None, :] < cc
        pos = starts[ctx_ids][:, None] + np.minimum(j[None, :],
                                                    np.maximum(cc - 1, 0))
        return order[pos], valid

    inv = np.float32(1.0 / WSCALE)
    x = np.asarray(x, np.float32) * inv
    u = np.asarray(u, np.float32) * inv

    # pre-mask the weight banks, scale into e3m4 range, quantize on host
    Am = (np.asarray(WA, np.float32) * np.float32(WSCALE)
          * np.asarray(adj_xx, np.float32)).astype(FP8)    # [C, S, S]
    Bm = (np.asarray(WB, np.float32) * np.float32(WSCALE)
          * np.asarray(adj_xu, np.float32)).astype(FP8)    # [C, A, S]

    geo, AL, tot_out, WF, _, _, _ = _group_geometry(S, A, Gs)

    in_maps = []
    scatter = []   # per core: list of (ctx_ids, gidx, valid) per group
    for k in range(N_CORES):
        wblob = np.empty((NG, 2, 128, CH, K, S), FP8)
        ablob = np.zeros((128, AL), BF16)
        sc = []
        for g, gg in enumerate(geo):
            G = gg["G"]
            ctx_ids = perm[(g * CT + np.arange(CT)) * N_CORES + k]
            gidx, valid = group_rows(ctx_ids, G)           # [CT, G]
            sc.append((ctx_ids, gidx, valid))
            XpT = x[gidx].transpose(0, 2, 1).astype(BF16)  # [CT, S, G]
            UpT = u[gidx].transpose(0, 2, 1).astype(BF16)  # [CT, A, G]
            wb = wblob[g]
            wb[..., 0, :] = Bm[ctx_ids].reshape(2, CH, 128, S) \
                .transpose(0, 2, 1, 3)
            wb[..., 1:, :] = Am[ctx_ids].reshape(2, CH, HS, 128, S) \
                .transpose(0, 3, 1, 2, 4)
            A3 = ablob[:, gg["off_a"]: gg["off_a"] + 2 * CH * K * G] \
                .reshape(128, 2, CH, K, G).transpose(1, 2, 3, 0, 4)
            A3[:, :, 0] = UpT.reshape(2, CH, 128, G)
            A3[:, :, 1:] = XpT.reshape(2, CH, HS, 128, G)
        in_maps.append({"wts": wblob.reshape(NG, 2, 128, WF),
                        "acts": ablob})
        scatter.append(sc)

    if _trace:
        _install_profile_shim()
    nc = _build_program(S, A, Gs)
    res = run_bass_kernel_spmd(nc, in_maps, core_ids=list(range(N_CORES)),
                               trace=_trace)

    # device output blob per group: [PSL, G, T, FF, S]; context c of the
    # group lives at bank t=c//CPT, partition slot sl=(c%CPT)//FF, free
    # half cf=c%FF -> row block [sl, :, t, cf, :].
    out_full = np.zeros((B, S), np.float32)
    for k, r in enumerate(res.results):
        v = np.asarray(r["out"]).astype(np.float32)
        for g, gg in enumerate(geo):
            G, T, FF, PSL, CPT = (gg["G"], gg["T"], gg["FF"], gg["PSL"],
                                  gg["CPT"])
            blk = v[gg["off_out"]: gg["off_out"] + PSL * G * T * FF * S] \
                .reshape(PSL, G, T, FF, S)
            ctx_ids, gidx, valid = scatter[k][g]
            for c in range(CT):
                t, r2 = divmod(c, CPT)
                sl, cf = divmod(r2, FF)
                rows = blk[sl, :, t, cf, :]                # [G, S]
                m = valid[c]
                out_full[gidx[c][m]] = rows[m]

    if _trace:
        return out_full, res

    return out_full


# revision 9
# speedup vs baseline: 2.2174x; 1.0013x over previous
"""Trainium2 Bass kernel for ContextHyperLinearSSM.

Computes out[b,:] = x[b,:] @ (WA[context[b]] * adj_xx) + u[b,:] @ (WB[context[b]] * adj_xu)

Strategy: shard the CONTEXT axis across the 8 cores (64 contexts each).
The host groups samples by context, masks the weight banks with the
adjacency masks, and quantizes the weights to fp8-e3m4 (x2^6 scale; the
inverse 2^-6 is folded into the bf16 activations — both scalings are exact
powers of two, so the only quantization error is the e3m4 weight rounding,
measured at 1.3e-2 absmax/scale against the fp32 reference).  Each core
streams its 64 contexts' weights from HBM exactly once and runs 3
accumulating mixed-dtype matmuls (bf16 stationary x fp8 moving) per
context.  Each sample's row is computed by exactly one core, so the
host-side unshard is a pure scatter.

Contexts are globally sorted by sample count and dealt round-robin to the
cores, so every core sees the same per-group padded size G_g (required:
one SPMD program serves all cores) and the padding tracks the count
distribution instead of the global max.  All activations ship in one
prefetched DMA; weights stream per half-group.  Groups are processed
largest first, so the tail group is the smallest; its weights are
additionally split into per-context DMAs so the final matmuls start as
soon as their slice of the last transfer lands.

All CT contexts of a group accumulate into T PSUM banks (two 64-aligned
partition slots x two free halves); one full-width copy per bank (vector
engine for bank 0, scalar for bank 1) drains PSUM to a bf16 staging tile
that DMAs out.
"""

import sys

sys.path.insert(0, "/opt/trn_rl_repo")

import ml_dtypes
import numpy as np

import concourse.bass as bass
import concourse.mybir as mybir
import concourse.tile as tile
from concourse import bacc
from concourse.bass_utils import run_bass_kernel_spmd

N_CORES = 8
CT = 8  # contexts per PSUM group
W_BUFS = 16  # buffer the whole weight stream: DMA never waits on the PE
WSCALE = 64.0  # 2^6: weights *= WSCALE (into e3m4 range), acts /= WSCALE

BF16 = ml_dtypes.bfloat16
FP8 = ml_dtypes.float8_e3m4


def _install_profile_shim():
    """Register the NTFF profile hook that trn_boot skips when
    antenv.axon_hooks is missing from the image (profiling only)."""
    import types
    if "antenv.axon_hooks" in sys.modules:
        return
    try:
        from trn_agent_boot.trn_boot import _ntff_profile_via_ctypes
        hook = _ntff_profile_via_ctypes("/opt/axon/libaxon_pjrt.so")
    except Exception:
        hook = None
    mod = types.ModuleType("antenv.axon_hooks")
    mod.get_axon_ntff_profile_hook = lambda: hook
    mod.set_axon_ntff_profile_hook = lambda h: None
    sys.modules["antenv.axon_hooks"] = mod


def _group_geometry(S, A, Gs):
    """Static per-group weight/act/psum geometry shared by host and device."""
    HS = S // 128
    K = HS + 1
    CH = CT // 2
    WF = CH * K * S          # weight elems per partition line per half-group
    geo = []
    off_a = 0                # acts offset, in elems per partition line
    off_out = 0
    for G in Gs:
        AF = CH * K * G
        FF = max(1, min(CT, 512 // S))
        PSL = 2 if G <= 64 else 1
        CPT = min(CT, PSL * FF)
        T = -(-CT // CPT)
        assert T * CPT == CT
        geo.append(dict(G=G, AF=AF, FF=FF, PSL=PSL, CPT=CPT, T=T,
                        off_a=off_a, off_out=off_out))
        off_a += 2 * AF
        off_out += PSL * G * T * FF * S
    return geo, off_a, off_out, WF, K, HS, CH


def _build_program(S, A, Gs):
    """Build the per-core Bass program for per-group sizes Gs."""
    f32 = mybir.dt.float32
    bf16 = mybir.dt.bfloat16
    fp8 = mybir.dt.float8e3
    nc = bacc.Bacc("TRN2", target_bir_lowering=False)

    geo, AL, tot_out, WF, K, HS, CH = _group_geometry(S, A, Gs)
    NG = len(Gs)
    assert S % 128 == 0 and A == 128

    wts = nc.dram_tensor("wts", [NG, 2, 128, WF], fp8,
                         kind="ExternalInput").ap()
    acts = nc.dram_tensor("acts", [128, AL], bf16,
                          kind="ExternalInput").ap()
    out = nc.dram_tensor("out", [tot_out], bf16, kind="ExternalOutput").ap()

    with tile.TileContext(nc) as tc:
        with (
            tc.tile_pool(name="a", bufs=1) as apool,
            tc.tile_pool(name="w", bufs=W_BUFS) as wpool,
            tc.tile_pool(name="o", bufs=8) as opool,
            tc.tile_pool(name="psum", bufs=8, space="PSUM") as psum,
        ):
            # all activations prefetched in one DMA (small: ~4KB/partition)
            # on the scalar/ACT ring so the sync ring starts weights at once
            at = apool.tile([128, AL], bf16)
            nc.scalar.dma_start(at[:], acts[:])

            for g, gg in enumerate(geo):
                G, FF, PSL, CPT, T = (gg["G"], gg["FF"], gg["PSL"],
                                      gg["CPT"], gg["T"])
                halves = []
                for hf in range(2):
                    wt = wpool.tile([128, WF], fp8, tag="wt",
                                    name=f"wt_{g}_{hf}")
                    if g == NG - 1:
                        # tail group: per-context weight DMAs so the last
                        # matmuls start as soon as their slice lands
                        for ci in range(CH):
                            nc.sync.dma_start(
                                wt[:, ci * K * S:(ci + 1) * K * S],
                                wts[g, hf, :, ci * K * S:(ci + 1) * K * S])
                    else:
                        nc.sync.dma_start(wt[:], wts[g, hf])
                    wv = wt[:].rearrange("p (c k s) -> p c k s", c=CH, k=K)
                    av = at[:, gg["off_a"] + hf * CH * K * G:
                            gg["off_a"] + (hf + 1) * CH * K * G] \
                        .rearrange("p (c k g) -> p c k g", c=CH, k=K)
                    halves.append((wv, av))

                ps_tiles = [psum.tile([128, FF * S], f32, tag="ps",
                                      name=f"ps_{g}_{t}")
                            for t in range(T)]
                for c in range(CT):
                    hf, ci = divmod(c, CH)
                    wv, av = halves[hf]
                    t, r2 = divmod(c, CPT)
                    sl, cf = divmod(r2, FF)
                    pslice = ps_tiles[t][sl * 64:sl * 64 + G,
                                         cf * S:cf * S + S]
                    for k in range(K):
                        nc.tensor.matmul(
                            pslice,
                            lhsT=av[:, ci, k, :],
                            rhs=wv[:, ci, k, :],
                            start=(k == 0), stop=(k == K - 1))
                out_sb = opool.tile([128, T, FF, S], bf16)
                for t in range(T):
                    # one full-width drain per bank (garbage rows are never
                    # DMA'd out); banks split across vector+scalar engines
                    eng = nc.vector.tensor_copy if t == 0 else nc.scalar.copy
                    eng(out_sb[:, t].rearrange("p f s -> p (f s)"),
                        ps_tiles[t][:, :])
                for sl in range(PSL):
                    dst = out[gg["off_out"] + sl * G * T * FF * S:
                              gg["off_out"] + (sl + 1) * G * T * FF * S]
                    nc.scalar.dma_start(
                        dst.rearrange("(gg l) -> gg l", gg=G),
                        out_sb[sl * 64:sl * 64 + G].rearrange(
                            "p t f s -> p (t f s)"))

    nc.compile()
    return nc


def kernel(x, u, WA, WB, adj_xx, adj_xu, context, _trace=False):
    B, S = x.shape
    _, A = u.shape
    C = WA.shape[0]
    assert C % N_CORES == 0
    CP = C // N_CORES
    assert CP % CT == 0
    NG = CP // CT
    HS = S // 128
    K = HS + 1
    CH = CT // 2

    # ---- host-side shard: count-sorted contexts, dealt round-robin ----
    context = np.asarray(context)
    cnt = np.bincount(context, minlength=C)
    perm = np.argsort(-cnt, kind="stable")          # contexts by count desc
    # context at global rank r -> core r%8, position r//8; group = pos//CT.
    # All cores share one program, so G_g is set by the chunk's global max
    # count = count at rank g*CT*N_CORES.
    Gs = []
    for g in range(NG):
        m = int(cnt[perm[g * CT * N_CORES]])
        Gs.append(max(2, ((m + 1) // 2) * 2))

    order = np.argsort(context, kind="stable")
    starts = np.zeros(C + 1, np.int64)
    starts[1:] = np.cumsum(cnt)

    def group_rows(ctx_ids, G):
        """gidx [len,G] sample indices (clamped) + valid mask."""
        j = np.arange(G)
        cc = cnt[ctx_ids][:, None]
        valid = j[None, :] < cc
        pos = starts[ctx_ids][:, None] + np.minimum(j[None, :],
                                                    np.maximum(cc - 1, 0))
        return order[pos], valid

    inv = np.float32(1.0 / WSCALE)
    x = np.asarray(x, np.float32) * inv
    u = np.asarray(u, np.float32) * inv

    # pre-mask the weight banks, scale into e3m4 range, quantize on host
    Am = (np.asarray(WA, np.float32) * np.float32(WSCALE)
          * np.asarray(adj_xx, np.float32)).astype(FP8)    # [C, S, S]
    Bm = (np.asarray(WB, np.float32) * np.float32(WSCALE)
          * np.asarray(adj_xu, np.float32)).astype(FP8)    # [C, A, S]

    geo, AL, tot_out, WF, _, _, _ = _group_geometry(S, A, Gs)

    in_maps = []
    scatter = []   # per core: list of (ctx_ids, gidx, valid) per group
    for k in range(N_CORES):
        wblob = np.empty((NG, 2, 128, CH, K, S), FP8)
        ablob = np.zeros((128, AL), BF16)
        sc = []
        for g, gg in enumerate(geo):
            G = gg["G"]
            ctx_ids = perm[(g * CT + np.arange(CT)) * N_CORES + k]
            gidx, valid = group_rows(ctx_ids, G)           # [CT, G]
            sc.append((ctx_ids, gidx, valid))
            XpT = x[gidx].transpose(0, 2, 1).astype(BF16)  # [CT, S, G]
            UpT = u[gidx].transpose(0, 2, 1).astype(BF16)  # [CT, A, G]
            wb = wblob[g]
            wb[..., 0, :] = Bm[ctx_ids].reshape(2, CH, 128, S) \
                .transpose(0, 2, 1, 3)
            wb[..., 1:, :] = Am[ctx_ids].reshape(2, CH, HS, 128, S) \
                .transpose(0, 3, 1, 2, 4)
            A3 = ablob[:, gg["off_a"]: gg["off_a"] + 2 * CH * K * G] \
                .reshape(128, 2, CH, K, G).transpose(1, 2, 3, 0, 4)
            A3[:, :, 0] = UpT.reshape(2, CH, 128, G)
            A3[:, :, 1:] = XpT.reshape(2, CH, HS, 128, G)
        in_maps.append({"wts": wblob.reshape(NG, 2, 128, WF),
                        "acts": ablob})
        scatter.append(sc)

    if _trace:
        _install_profile_shim()
    nc = _build_program(S, A, Gs)
    res = run_bass_kernel_spmd(nc, in_maps, core_ids=list(range(N_CORES)),
                               trace=_trace)

    # device output blob per group: [PSL, G, T, FF, S]; context c of the
    # group lives at bank t=c//CPT, partition slot sl=(c%CPT)//FF, free
    # half cf=c%FF -> row block [sl, :, t, cf, :].
    out_full = np.zeros((B, S), np.float32)
    for k, r in enumerate(res.results):
        v = np.asarray(r["out"]).astype(np.float32)
        for g, gg in enumerate(geo):
            G, T, FF, PSL, CPT = (gg["G"], gg["T"], gg["FF"], gg["PSL"],
                                  gg["CPT"])
            blk = v[gg["off_out"]: gg["off_out"] + PSL * G * T * FF * S] \
                .reshape(PSL, G, T, FF, S)
            ctx_ids, gidx, valid = scatter[k][g]
            for c in range(CT):
                t, r2 = divmod(c, CPT)
                sl, cf = divmod(r2, FF)
                rows = blk[sl, :, t, cf, :]                # [G, S]
                m = valid[c]
                out_full[gidx[c][m]] = rows[m]

    if _trace:
        return out_full, res

    return out_full


# revision 14
# speedup vs baseline: 2.4818x; 1.1192x over previous
"""Trainium2 Bass kernel for ContextHyperLinearSSM.

Computes out[b,:] = x[b,:] @ (WA[context[b]] * adj_xx) + u[b,:] @ (WB[context[b]] * adj_xu)

Strategy: shard the CONTEXT axis across the 8 cores (64 contexts each).
The host groups samples by context, masks the weight banks with the
adjacency masks, and quantizes the weights to fp8-e3m4 (x2^6 scale; the
inverse 2^-6 is folded into the bf16 activations — both scalings are exact
powers of two, so the only quantization error is the e3m4 weight rounding,
measured at 1.3e-2 absmax/scale against the fp32 reference).  Each core
streams its 64 contexts' weights from HBM exactly once and runs 3
accumulating mixed-dtype matmuls (bf16 stationary x fp8 moving) per
context.  Each sample's row is computed by exactly one core, so the
host-side unshard is a pure scatter.

Contexts are globally sorted by sample count and dealt round-robin to the
cores, so every core sees the same per-group padded size G_g (required:
one SPMD program serves all cores) and the padding tracks the count
distribution instead of the global max.  All activations ship in one
prefetched DMA; weights stream per half-group.  Groups are processed
largest first, so the tail group is the smallest; its weights are
additionally split into per-context DMAs so the final matmuls start as
soon as their slice of the last transfer lands.

All CT contexts of a group accumulate into T PSUM banks (two 64-aligned
partition slots x two free halves); one full-width copy per bank (vector
engine for bank 0, scalar for bank 1) drains PSUM to a bf16 staging tile
that DMAs out.
"""

import sys

sys.path.insert(0, "/opt/trn_rl_repo")

import ml_dtypes
import numpy as np

import concourse.bass as bass
import concourse.mybir as mybir
import concourse.tile as tile
from concourse import bacc
from concourse.bass_utils import run_bass_kernel_spmd

N_CORES = 8
CT = 8  # contexts per PSUM group
W_BUFS = 8  # one tile per group: DMA never waits on the PE
WSCALE = 64.0  # 2^6: weights *= WSCALE (into e3m4 range), acts /= WSCALE

BF16 = ml_dtypes.bfloat16
FP8 = ml_dtypes.float8_e3m4


def _install_profile_shim():
    """Register the NTFF profile hook that trn_boot skips when
    antenv.axon_hooks is missing from the image (profiling only)."""
    import types
    if "antenv.axon_hooks" in sys.modules:
        return
    try:
        from trn_agent_boot.trn_boot import _ntff_profile_via_ctypes
        hook = _ntff_profile_via_ctypes("/opt/axon/libaxon_pjrt.so")
    except Exception:
        hook = None
    mod = types.ModuleType("antenv.axon_hooks")
    mod.get_axon_ntff_profile_hook = lambda: hook
    mod.set_axon_ntff_profile_hook = lambda h: None
    sys.modules["antenv.axon_hooks"] = mod


def _group_geometry(S, A, Gs):
    """Static per-group weight/act/psum geometry shared by host and device."""
    HS = S // 128
    K = HS + 1
    CH = CT // 2
    WF = CH * K * S          # weight elems per partition line per half-group
    geo = []
    off_a = 0                # acts offset, in elems per partition line
    off_out = 0
    for G in Gs:
        AF = CH * K * G
        FF = max(1, min(CT, 512 // S))
        PSL = 2 if G <= 64 else 1
        CPT = min(CT, PSL * FF)
        T = -(-CT // CPT)
        assert T * CPT == CT
        geo.append(dict(G=G, AF=AF, FF=FF, PSL=PSL, CPT=CPT, T=T,
                        off_a=off_a, off_out=off_out))
        off_a += 2 * AF
        off_out += PSL * G * T * FF * S
    return geo, off_a, off_out, WF, K, HS, CH


def _build_program(S, A, Gs):
    """Build the per-core Bass program for per-group sizes Gs."""
    f32 = mybir.dt.float32
    bf16 = mybir.dt.bfloat16
    fp8 = mybir.dt.float8e3
    nc = bacc.Bacc("TRN2", target_bir_lowering=False)

    geo, AL, tot_out, WF, K, HS, CH = _group_geometry(S, A, Gs)
    NG = len(Gs)
    assert S % 128 == 0 and A == 128

    wts = nc.dram_tensor("wts", [NG, 128, 2 * WF], fp8,
                         kind="ExternalInput").ap()
    acts = nc.dram_tensor("acts", [128, AL], bf16,
                          kind="ExternalInput").ap()
    out = nc.dram_tensor("out", [tot_out], bf16, kind="ExternalOutput").ap()

    with tile.TileContext(nc) as tc:
        with (
            tc.tile_pool(name="a", bufs=1) as apool,
            tc.tile_pool(name="w", bufs=W_BUFS) as wpool,
            tc.tile_pool(name="o", bufs=8) as opool,
            tc.tile_pool(name="psum", bufs=8, space="PSUM") as psum,
        ):
            # all activations prefetched in one DMA (small: ~4KB/partition)
            # on the scalar/ACT ring so the sync ring starts weights at once
            at = apool.tile([128, AL], bf16)
            nc.scalar.dma_start(at[:], acts[:])

            for g, gg in enumerate(geo):
                G, FF, PSL, CPT, T = (gg["G"], gg["FF"], gg["PSL"],
                                      gg["CPT"], gg["T"])
                # one DMA per group (128 descriptors of 2*WF=6KB lines),
                # groups alternating across the two HWDGE rings so issue
                # cost and completion-sem lanes don't serialize the stream
                ring = nc.sync if g % 2 == 0 else nc.scalar
                wt = wpool.tile([128, 2 * WF], fp8, tag="wt", name=f"wt_{g}")
                if g == NG - 1:
                    # tail group: per-context weight DMAs so the last
                    # matmuls start as soon as their slice lands
                    for hf in range(2):
                        for ci in range(CH):
                            lo = hf * WF + ci * K * S
                            ring.dma_start(wt[:, lo:lo + K * S],
                                           wts[g, :, lo:lo + K * S])
                else:
                    ring.dma_start(wt[:], wts[g])
                halves = []
                for hf in range(2):
                    wv = wt[:, hf * WF:(hf + 1) * WF] \
                        .rearrange("p (c k s) -> p c k s", c=CH, k=K)
                    av = at[:, gg["off_a"] + hf * CH * K * G:
                            gg["off_a"] + (hf + 1) * CH * K * G] \
                        .rearrange("p (c k g) -> p c k g", c=CH, k=K)
                    halves.append((wv, av))

                ps_tiles = [psum.tile([128, FF * S], f32, tag="ps",
                                      name=f"ps_{g}_{t}")
                            for t in range(T)]
                for c in range(CT):
                    hf, ci = divmod(c, CH)
                    wv, av = halves[hf]
                    t, r2 = divmod(c, CPT)
                    sl, cf = divmod(r2, FF)
                    pslice = ps_tiles[t][sl * 64:sl * 64 + G,
                                         cf * S:cf * S + S]
                    for k in range(K):
                        nc.tensor.matmul(
                            pslice,
                            lhsT=av[:, ci, k, :],
                            rhs=wv[:, ci, k, :],
                            start=(k == 0), stop=(k == K - 1))
                out_sb = opool.tile([128, T, FF, S], bf16)
                for t in range(T):
                    # one full-width drain per bank (garbage rows are never
                    # DMA'd out); banks split across vector+scalar engines
                    eng = nc.vector.tensor_copy if t == 0 else nc.scalar.copy
                    eng(out_sb[:, t].rearrange("p f s -> p (f s)"),
                        ps_tiles[t][:, :])
                for sl in range(PSL):
                    dst = out[gg["off_out"] + sl * G * T * FF * S:
                              gg["off_out"] + (sl + 1) * G * T * FF * S]
                    nc.sync.dma_start(
                        dst.rearrange("(gg l) -> gg l", gg=G),
                        out_sb[sl * 64:sl * 64 + G].rearrange(
                            "p t f s -> p (t f s)"))

    nc.compile()
    return nc


def kernel(x, u, WA, WB, adj_xx, adj_xu, context, _trace=False):
    B, S = x.shape
    _, A = u.shape
    C = WA.shape[0]
    assert C % N_CORES == 0
    CP = C // N_CORES
    assert CP % CT == 0
    NG = CP // CT
    HS = S // 128
    K = HS + 1
    CH = CT // 2

    # ---- host-side shard: count-sorted contexts, dealt round-robin ----
    context = np.asarray(context)
    cnt = np.bincount(context, minlength=C)
    perm = np.argsort(-cnt, kind="stable")          # contexts by count desc
    # context at global rank r -> core r%8, position r//8; group = pos//CT.
    # All cores share one program, so G_g is set by the chunk's global max
    # count = count at rank g*CT*N_CORES.
    Gs = []
    for g in range(NG):
        m = int(cnt[perm[g * CT * N_CORES]])
        Gs.append(max(2, ((m + 1) // 2) * 2))

    order = np.argsort(context, kind="stable")
    starts = np.zeros(C + 1, np.int64)
    starts[1:] = np.cumsum(cnt)

    def group_rows(ctx_ids, G):
        """gidx [len,G] sample indices (clamped) + valid mask."""
        j = np.arange(G)
        cc = cnt[ctx_ids][:, None]
        valid = j[None, :] < cc
        pos = starts[ctx_ids][:, None] + np.minimum(j[None, :],
                                                    np.maximum(cc - 1, 0))
        return order[pos], valid

    inv = np.float32(1.0 / WSCALE)
    x = np.asarray(x, np.float32) * inv
    u = np.asarray(u, np.float32) * inv

    # pre-mask the weight banks, scale into e3m4 range, quantize on host
    Am = (np.asarray(WA, np.float32) * np.float32(WSCALE)
          * np.asarray(adj_xx, np.float32)).astype(FP8)    # [C, S, S]
    Bm = (np.asarray(WB, np.float32) * np.float32(WSCALE)
          * np.asarray(adj_xu, np.float32)).astype(FP8)    # [C, A, S]

    geo, AL, tot_out, WF, _, _, _ = _group_geometry(S, A, Gs)

    in_maps = []
    scatter = []   # per core: list of (ctx_ids, gidx, valid) per group
    for k in range(N_CORES):
        wblob = np.empty((NG, 128, 2, CH, K, S), FP8)
        ablob = np.zeros((128, AL), BF16)
        sc = []
        for g, gg in enumerate(geo):
            G = gg["G"]
            ctx_ids = perm[(g * CT + np.arange(CT)) * N_CORES + k]
            gidx, valid = group_rows(ctx_ids, G)           # [CT, G]
            sc.append((ctx_ids, gidx, valid))
            XpT = x[gidx].transpose(0, 2, 1).astype(BF16)  # [CT, S, G]
            UpT = u[gidx].transpose(0, 2, 1).astype(BF16)  # [CT, A, G]
            wb = wblob[g].transpose(1, 2, 0, 3, 4)         # [2,CH,128,K,S]
            wb[..., 0, :] = Bm[ctx_ids].reshape(2, CH, 128, S)
            wb[..., 1:, :] = Am[ctx_ids].reshape(2, CH, HS, 128, S) \
                .transpose(0, 1, 3, 2, 4)
            A3 = ablob[:, gg["off_a"]: gg["off_a"] + 2 * CH * K * G] \
                .reshape(128, 2, CH, K, G).transpose(1, 2, 3, 0, 4)
            A3[:, :, 0] = UpT.reshape(2, CH, 128, G)
            A3[:, :, 1:] = XpT.reshape(2, CH, HS, 128, G)
        in_maps.append({"wts": wblob.reshape(NG, 128, 2 * WF),
                        "acts": ablob})
        scatter.append(sc)

    if _trace:
        _install_profile_shim()
    nc = _build_program(S, A, Gs)
    res = run_bass_kernel_spmd(nc, in_maps, core_ids=list(range(N_CORES)),
                               trace=_trace)

    # device output blob per group: [PSL, G, T, FF, S]; context c of the
    # group lives at bank t=c//CPT, partition slot sl=(c%CPT)//FF, free
    # half cf=c%FF -> row block [sl, :, t, cf, :].
    out_full = np.zeros((B, S), np.float32)
    for k, r in enumerate(res.results):
        v = np.asarray(r["out"]).astype(np.float32)
        for g, gg in enumerate(geo):
            G, T, FF, PSL, CPT = (gg["G"], gg["T"], gg["FF"], gg["PSL"],
                                  gg["CPT"])
            blk = v[gg["off_out"]: gg["off_out"] + PSL * G * T * FF * S] \
                .reshape(PSL, G, T, FF, S)
            ctx_ids, gidx, valid = scatter[k][g]
            for c in range(CT):
                t, r2 = divmod(c, CPT)
                sl, cf = divmod(r2, FF)
                rows = blk[sl, :, t, cf, :]                # [G, S]
                m = valid[c]
                out_full[gidx[c][m]] = rows[m]

    if _trace:
        return out_full, res

    return out_full


# revision 17
# speedup vs baseline: 2.5778x; 1.0387x over previous
"""Trainium2 Bass kernel for ContextHyperLinearSSM.

Computes out[b,:] = x[b,:] @ (WA[context[b]] * adj_xx) + u[b,:] @ (WB[context[b]] * adj_xu)

Strategy: shard the CONTEXT axis across the 8 cores (64 contexts each).
The host groups samples by context, masks the weight banks with the
adjacency masks, and quantizes the weights to fp8-e3m4 (x2^6 scale; the
inverse 2^-6 is folded into the bf16 activations — both scalings are exact
powers of two, so the only quantization error is the e3m4 weight rounding,
measured at 1.3e-2 absmax/scale against the fp32 reference).  Each core
streams its 64 contexts' weights from HBM exactly once and runs 3
accumulating mixed-dtype matmuls (bf16 stationary x fp8 moving) per
context.  Each sample's row is computed by exactly one core, so the
host-side unshard is a pure scatter.

Contexts are globally sorted by sample count and dealt round-robin to the
cores, so every core sees the same per-group padded size G_g (required:
one SPMD program serves all cores) and the padding tracks the count
distribution.  All activations ship in one prefetched DMA; weights stream
one merged DMA per group of CT contexts, groups alternating between the
two HWDGE rings (sync/scalar) so descriptor-generation cost and
completion-semaphore reuse never serialize the stream.  Group 0 is split
into per-half DMAs on both rings (shorter pipeline-fill latency) and the
final, smallest group into per-context DMAs (matmuls start as soon as
their slice of the last transfer lands).

Compute: all CT contexts of a group accumulate into ONE PSUM bank packed
as 4 x 32-aligned partition slots x 2 free halves; matmuls are emitted in
k-major waves cycling the four 128x32 column tiles of the PE array so
four matmuls execute concurrently.  One full-width copy per group (vector
and scalar engines alternating) drains PSUM into a shared bf16 staging
tile, which is flushed by a few consolidated multi-group DMAs.
"""

import sys

sys.path.insert(0, "/opt/trn_rl_repo")

import ml_dtypes
import numpy as np

import concourse.bass as bass
import concourse.mybir as mybir
import concourse.tile as tile
from concourse import bacc
from concourse.bass_utils import run_bass_kernel_spmd

N_CORES = 8
CT = 8  # contexts per PSUM group
WSCALE = 64.0  # 2^6: weights *= WSCALE (into e3m4 range), acts /= WSCALE

BF16 = ml_dtypes.bfloat16
FP8 = ml_dtypes.float8_e3m4


def _install_profile_shim():
    """Register the NTFF profile hook that trn_boot skips when
    antenv.axon_hooks is missing from the image (profiling only)."""
    import types
    if "antenv.axon_hooks" in sys.modules:
        return
    try:
        from trn_agent_boot.trn_boot import _ntff_profile_via_ctypes
        hook = _ntff_profile_via_ctypes("/opt/axon/libaxon_pjrt.so")
    except Exception:
        hook = None
    mod = types.ModuleType("antenv.axon_hooks")
    mod.get_axon_ntff_profile_hook = lambda: hook
    mod.set_axon_ntff_profile_hook = lambda h: None
    sys.modules["antenv.axon_hooks"] = mod


def _geometry(S, A, Gs):
    """Static geometry shared by host and device.

    PSUM packing: context c of a group -> bank t=c//CPT, partition slot
    sl=(c%CPT)%NSL (SLP-aligned), free half cf=(c%CPT)//NSL.
    """
    HS = S // 128
    K = HS + 1
    CH = CT // 2
    WF = CH * K * S
    FF = max(1, min(CT, 512 // S))
    SLP = 32 if max(Gs) <= 32 else 64
    NSL = 128 // SLP
    CPT = min(CT, NSL * FF)
    T = -(-CT // CPT)
    assert T * CPT == CT
    NG = len(Gs)
    # out-staging splits: [0,NG/2), [NG/2,NG-1), [NG-1,NG) — early flushes
    # plus a tiny final one (groups are sorted largest-first)
    splits = [(0, NG // 2), (NG // 2, NG - 1), (NG - 1, NG)]
    splits = [(a, b) for a, b in splits if b > a]
    off_a = 0
    offs_a = []
    for G in Gs:
        offs_a.append(off_a)
        off_a += 2 * CH * K * G
    OW = T * FF * S  # staging elems per partition line per group
    off_o = 0
    offs_o = []
    for a, b in splits:
        GH = max(Gs[a:b])
        offs_o.append(off_o)
        off_o += NSL * GH * (b - a) * OW
    return dict(HS=HS, K=K, CH=CH, WF=WF, FF=FF, SLP=SLP, NSL=NSL,
                CPT=CPT, T=T, NG=NG, splits=splits, offs_a=offs_a,
                AL=off_a, OW=OW, offs_o=offs_o, OL=off_o)


def _build_program(S, A, Gs):
    """Build the per-core Bass program for per-group sizes Gs."""
    f32 = mybir.dt.float32
    bf16 = mybir.dt.bfloat16
    fp8 = mybir.dt.float8e3
    nc = bacc.Bacc("TRN2", target_bir_lowering=False)

    geo = _geometry(S, A, Gs)
    K, CH, WF, FF = geo["K"], geo["CH"], geo["WF"], geo["FF"]
    SLP, NSL, CPT, T = geo["SLP"], geo["NSL"], geo["CPT"], geo["T"]
    NG, OW = geo["NG"], geo["OW"]
    assert S % 128 == 0 and A == 128

    wts = nc.dram_tensor("wts", [NG, 128, 2 * WF], fp8,
                         kind="ExternalInput").ap()
    acts = nc.dram_tensor("acts", [128, geo["AL"]], bf16,
                          kind="ExternalInput").ap()
    out = nc.dram_tensor("out", [geo["OL"]], bf16,
                         kind="ExternalOutput").ap()

    with tile.TileContext(nc) as tc:
        with (
            tc.tile_pool(name="a", bufs=1) as apool,
            tc.tile_pool(name="w", bufs=NG) as wpool,
            tc.tile_pool(name="o", bufs=1) as opool,
            tc.tile_pool(name="psum", bufs=8, space="PSUM") as psum,
        ):
            # all activations prefetched in one DMA (small: ~4KB/partition)
            at = apool.tile([128, geo["AL"]], bf16)
            nc.scalar.dma_start(at[:], acts[:])
            # shared output staging tile, flushed by consolidated DMAs
            os_t = opool.tile([128, NG * OW], bf16)

            for g in range(NG):
                G = Gs[g]
                ring = nc.sync if g % 2 == 0 else nc.scalar
                wt = wpool.tile([128, 2 * WF], fp8, tag="wt", name=f"wt_{g}")
                if g == 0:
                    # pipeline fill: one half per ring, in parallel
                    nc.sync.dma_start(wt[:, :WF], wts[0, :, :WF])
                    nc.scalar.dma_start(wt[:, WF:], wts[0, :, WF:])
                elif g == NG - 1:
                    # tail group: per-context weight DMAs so the last
                    # matmuls start as soon as their slice lands
                    for hf in range(2):
                        for ci in range(CH):
                            lo = hf * WF + ci * K * S
                            ring.dma_start(wt[:, lo:lo + K * S],
                                           wts[g, :, lo:lo + K * S])
                else:
                    ring.dma_start(wt[:], wts[g])

                def views(c):
                    hf, ci = divmod(c, CH)
                    wv = wt[:, hf * WF + ci * K * S:
                            hf * WF + (ci + 1) * K * S] \
                        .rearrange("p (k s) -> p k s", k=K)
                    a0 = geo["offs_a"][g] + (hf * CH + ci) * K * G
                    av = at[:, a0:a0 + K * G] \
                        .rearrange("p (k g) -> p k g", k=K)
                    return wv, av

                ps_tiles = [psum.tile([128, FF * S], f32, tag="ps",
                                      name=f"ps_{g}_{t}")
                            for t in range(T)]
                # consecutive contexts cycle the NSL column tiles of the
                # PE array so their streams execute concurrently; each
                # context's K accumulating matmuls stay adjacent (the
                # LDWEIGHTS lookahead only tracks one shadow per tile)
                for c in range(CT):
                    wv, av = views(c)
                    t, r2 = divmod(c, CPT)
                    sl, cf = r2 % NSL, r2 // NSL
                    pslice = ps_tiles[t][sl * SLP:sl * SLP + G,
                                         cf * S:cf * S + S]
                    for k in range(K):
                        nc.tensor.matmul(
                            pslice,
                            lhsT=av[:, k, :],
                            rhs=wv[:, k, :],
                            start=(k == 0), stop=(k == K - 1),
                            tile_position=(0, sl * SLP))
                for t in range(T):
                    # one full-width drain per bank (garbage rows are never
                    # DMA'd out); engines alternate per group
                    eng = (nc.vector.tensor_copy if g % 2 == 0
                           else nc.scalar.copy)
                    eng(os_t[:, (g * T + t) * FF * S:
                             (g * T + t + 1) * FF * S],
                        ps_tiles[t][:, :])

            # consolidated output flushes: per split x partition slot
            for si, (a, b) in enumerate(geo["splits"]):
                GH = max(Gs[a:b])
                W = (b - a) * OW
                for sl in range(NSL):
                    dst = out[geo["offs_o"][si] + sl * GH * W:
                              geo["offs_o"][si] + (sl + 1) * GH * W]
                    ring = nc.sync if (si * NSL + sl) % 2 == 0 else nc.scalar
                    ring.dma_start(
                        dst.rearrange("(gh w) -> gh w", gh=GH),
                        os_t[sl * SLP:sl * SLP + GH, a * OW:b * OW])

    nc.compile()
    return nc


def kernel(x, u, WA, WB, adj_xx, adj_xu, context, _trace=False):
    B, S = x.shape
    _, A = u.shape
    C = WA.shape[0]
    assert C % N_CORES == 0
    CP = C // N_CORES
    assert CP % CT == 0
    NG = CP // CT

    # ---- host-side shard: count-sorted contexts, dealt round-robin ----
    context = np.asarray(context)
    cnt = np.bincount(context, minlength=C)
    perm = np.argsort(-cnt, kind="stable")          # contexts by count desc
    # context at global rank r -> core r%8, position r//8; group = pos//CT.
    # All cores share one program, so G_g is set by the chunk's global max
    # count = count at rank g*CT*N_CORES.
    Gs = []
    for g in range(NG):
        m = int(cnt[perm[g * CT * N_CORES]])
        Gs.append(max(2, ((m + 1) // 2) * 2))

    geo = _geometry(S, A, Gs)
    HS, K, CH, WF = geo["HS"], geo["K"], geo["CH"], geo["WF"]
    FF, SLP, NSL, CPT, T, OW = (geo["FF"], geo["SLP"], geo["NSL"],
                                geo["CPT"], geo["T"], geo["OW"])

    order = np.argsort(context, kind="stable")
    starts = np.zeros(C + 1, np.int64)
    starts[1:] = np.cumsum(cnt)

    def group_rows(ctx_ids, G):
        """gidx [len,G] sample indices (clamped) + valid mask."""
        j = np.arange(G)
        cc = cnt[ctx_ids][:, None]
        valid = j[None, :] < cc
        pos = starts[ctx_ids][:, None] + np.minimum(j[None, :],
                                                    np.maximum(cc - 1, 0))
        return order[pos], valid

    inv = np.float32(1.0 / WSCALE)
    x = np.asarray(x, np.float32) * inv
    u = np.asarray(u, np.float32) * inv

    # pre-mask the weight banks, scale into e3m4 range, quantize on host
    Am = (np.asarray(WA, np.float32) * np.float32(WSCALE)
          * np.asarray(adj_xx, np.float32)).astype(FP8)    # [C, S, S]
    Bm = (np.asarray(WB, np.float32) * np.float32(WSCALE)
          * np.asarray(adj_xu, np.float32)).astype(FP8)    # [C, A, S]

    in_maps = []
    scatter = []   # per core: list of (ctx_ids, gidx, valid) per group
    for k in range(N_CORES):
        wblob = np.empty((NG, 128, 2, CH, K, S), FP8)
        ablob = np.zeros((128, geo["AL"]), BF16)
        sc = []
        for g in range(NG):
            G = Gs[g]
            ctx_ids = perm[(g * CT + np.arange(CT)) * N_CORES + k]
            gidx, valid = group_rows(ctx_ids, G)           # [CT, G]
            sc.append((ctx_ids, gidx, valid))
            XpT = x[gidx].transpose(0, 2, 1).astype(BF16)  # [CT, S, G]
            UpT = u[gidx].transpose(0, 2, 1).astype(BF16)  # [CT, A, G]
            wb = wblob[g].transpose(1, 2, 0, 3, 4)         # [2,CH,128,K,S]
            wb[..., 0, :] = Bm[ctx_ids].reshape(2, CH, 128, S)
            wb[..., 1:, :] = Am[ctx_ids].reshape(2, CH, HS, 128, S) \
                .transpose(0, 1, 3, 2, 4)
            A3 = ablob[:, geo["offs_a"][g]:
                       geo["offs_a"][g] + 2 * CH * K * G] \
                .reshape(128, 2, CH, K, G).transpose(1, 2, 3, 0, 4)
            A3[:, :, 0] = UpT.reshape(2, CH, 128, G)
            A3[:, :, 1:] = XpT.reshape(2, CH, HS, 128, G)
        in_maps.append({"wts": wblob.reshape(NG, 128, 2 * WF),
                        "acts": ablob})
        scatter.append(sc)

    if _trace:
        _install_profile_shim()
    nc = _build_program(S, A, Gs)
    res = run_bass_kernel_spmd(nc, in_maps, core_ids=list(range(N_CORES)),
                               trace=_trace)

    # unscatter: split si block is [NSL, GH, b-a groups, T, FF, S]
    out_full = np.zeros((B, S), np.float32)
    for k, r in enumerate(res.results):
        v = np.asarray(r["out"]).astype(np.float32)
        for si, (a, b) in enumerate(geo["splits"]):
            GH = max(Gs[a:b])
            blk = v[geo["offs_o"][si]:
                    geo["offs_o"][si] + NSL * GH * (b - a) * OW] \
                .reshape(NSL, GH, b - a, T, FF, S)
            for g in range(a, b):
                ctx_ids, gidx, valid = scatter[k][g]
                for c in range(CT):
                    t, r2 = divmod(c, CPT)
                    sl, cf = r2 % NSL, r2 // NSL
                    rows = blk[sl, :Gs[g], g - a, t, cf, :]  # [G, S]
                    m = valid[c]
                    out_full[gidx[c][m]] = rows[m]

    if _trace:
        return out_full, res

    return out_full


# revision 22
# speedup vs baseline: 2.8863x; 1.1197x over previous
"""Trainium2 Bass kernel for ContextHyperLinearSSM.

Computes out[b,:] = x[b,:] @ (WA[context[b]] * adj_xx) + u[b,:] @ (WB[context[b]] * adj_xu)

Strategy: shard the CONTEXT axis across the 8 cores (64 contexts each).
The host groups samples by context, masks the weight banks with the
adjacency masks, and quantizes the weights to fp8-e3m4 (x2^6 scale; the
inverse 2^-6 is folded into the bf16 activations — both scalings are exact
powers of two, so the only quantization error is the e3m4 weight rounding,
measured at 1.3e-2 absmax/scale against the fp32 reference).  Each core
streams its 64 contexts' weights from HBM exactly once and runs 3
accumulating mixed-dtype matmuls (bf16 stationary x fp8 moving) per
context.  Each sample's row is computed by exactly one core, so the
host-side unshard is a pure scatter.

Contexts are globally sorted by sample count and dealt round-robin to the
cores, so every core sees the same per-group padded size G_g (required:
one SPMD program serves all cores) and the padding tracks the count
distribution.  All activations ship in one prefetched DMA; weights stream
one merged DMA per group of CT contexts, groups alternating between the
two HWDGE rings (sync/scalar) so descriptor-generation cost and
completion-semaphore reuse never serialize the stream.  Group 0 is split
into per-half DMAs on both rings (shorter pipeline-fill latency) and the
final, smallest group into per-context DMAs (matmuls start as soon as
their slice of the last transfer lands).

Compute: all CT contexts of a group accumulate into ONE PSUM bank packed
as 4 x 32-aligned partition slots x 2 free halves; matmuls are emitted in
k-major waves cycling the four 128x32 column tiles of the PE array so
four matmuls execute concurrently.  One full-width copy per group (vector
and scalar engines alternating) drains PSUM into a shared bf16 staging
tile, which is flushed by a few consolidated multi-group DMAs.
"""

import sys

sys.path.insert(0, "/opt/trn_rl_repo")

import ml_dtypes
import numpy as np

import concourse.bass as bass
import concourse.mybir as mybir
import concourse.tile as tile
from concourse import bacc
from concourse.bass_utils import run_bass_kernel_spmd

N_CORES = 8
CT = 8  # contexts per PSUM group
WSCALE = 64.0  # 2^6: weights *= WSCALE (into e3m4 range), acts /= WSCALE

BF16 = ml_dtypes.bfloat16
FP8 = ml_dtypes.float8_e3m4


def _install_profile_shim():
    """Register the NTFF profile hook that trn_boot skips when
    antenv.axon_hooks is missing from the image (profiling only)."""
    import types
    if "antenv.axon_hooks" in sys.modules:
        return
    try:
        from trn_agent_boot.trn_boot import _ntff_profile_via_ctypes
        hook = _ntff_profile_via_ctypes("/opt/axon/libaxon_pjrt.so")
    except Exception:
        hook = None
    mod = types.ModuleType("antenv.axon_hooks")
    mod.get_axon_ntff_profile_hook = lambda: hook
    mod.set_axon_ntff_profile_hook = lambda h: None
    sys.modules["antenv.axon_hooks"] = mod


def _geometry(S, A, Gs):
    """Static geometry shared by host and device.

    PSUM packing: context c of a group -> bank t=c//CPT, partition slot
    sl=(c%CPT)%NSL (SLP-aligned), free half cf=(c%CPT)//NSL.
    """
    HS = S // 128
    K = HS + 1
    CH = CT // 2
    WF = CH * K * S
    FF = max(1, min(CT, 512 // S))
    SLP = 32 if max(Gs) <= 32 else 64
    NSL = 128 // SLP
    CPT = min(CT, NSL * FF)
    T = -(-CT // CPT)
    assert T * CPT == CT
    NG = len(Gs)
    # out-staging splits: [0,NG/2), [NG/2,NG-1), [NG-1,NG) — early flushes
    # plus a tiny final one (groups are sorted largest-first)
    splits = [(0, NG // 2), (NG // 2, NG - 1), (NG - 1, NG)]
    splits = [(a, b) for a, b in splits if b > a]
    off_a = 0
    offs_a = []
    for G in Gs:
        offs_a.append(off_a)
        off_a += 2 * CH * K * G
    OW = T * FF * S  # staging elems per partition line per group
    return dict(HS=HS, K=K, CH=CH, WF=WF, FF=FF, SLP=SLP, NSL=NSL,
                CPT=CPT, T=T, NG=NG, splits=splits, offs_a=offs_a,
                AL=off_a, OW=OW, OL=128 * NG * OW)


def _build_program(S, A, Gs):
    """Build the per-core Bass program for per-group sizes Gs."""
    f32 = mybir.dt.float32
    bf16 = mybir.dt.bfloat16
    fp8 = mybir.dt.float8e3
    nc = bacc.Bacc("TRN2", target_bir_lowering=False)

    geo = _geometry(S, A, Gs)
    K, CH, WF, FF = geo["K"], geo["CH"], geo["WF"], geo["FF"]
    SLP, NSL, CPT, T = geo["SLP"], geo["NSL"], geo["CPT"], geo["T"]
    NG, OW = geo["NG"], geo["OW"]
    assert S % 128 == 0 and A == 128

    wts = nc.dram_tensor("wts", [NG, 128, 2 * WF], fp8,
                         kind="ExternalInput").ap()
    acts = nc.dram_tensor("acts", [128, geo["AL"]], bf16,
                          kind="ExternalInput").ap()
    out = nc.dram_tensor("out", [geo["OL"]], bf16,
                         kind="ExternalOutput").ap()

    with tile.TileContext(nc) as tc:
        with (
            tc.tile_pool(name="a", bufs=1) as apool,
            tc.tile_pool(name="w", bufs=NG) as wpool,
            tc.tile_pool(name="o", bufs=1) as opool,
            tc.tile_pool(name="psum", bufs=8, space="PSUM") as psum,
        ):
            # all activations prefetched in one DMA (small: ~4KB/partition)
            at = apool.tile([128, geo["AL"]], bf16)
            nc.scalar.dma_start(at[:], acts[:])
            # shared output staging tile, flushed by consolidated DMAs
            os_t = opool.tile([128, NG * OW], bf16)

            for g in range(NG):
                G = Gs[g]
                ring = nc.sync if g % 2 == 0 else nc.scalar
                wt = wpool.tile([128, 2 * WF], fp8, tag="wt", name=f"wt_{g}")
                if g == 0:
                    # pipeline fill: one half per ring, in parallel
                    nc.sync.dma_start(wt[:, :WF], wts[0, :, :WF])
                    nc.scalar.dma_start(wt[:, WF:], wts[0, :, WF:])
                elif g == NG - 1:
                    # tail group: per-context weight DMAs (alternating
                    # rings) so the last matmuls start as soon as their
                    # slice lands and completion receipts overlap
                    for hf in range(2):
                        for ci in range(CH):
                            lo = hf * WF + ci * K * S
                            r = nc.sync if (hf * CH + ci) % 2 == 0 \
                                else nc.scalar
                            r.dma_start(wt[:, lo:lo + K * S],
                                        wts[g, :, lo:lo + K * S])
                else:
                    ring.dma_start(wt[:], wts[g])

                def views(c):
                    hf, ci = divmod(c, CH)
                    wv = wt[:, hf * WF + ci * K * S:
                            hf * WF + (ci + 1) * K * S] \
                        .rearrange("p (k s) -> p k s", k=K)
                    a0 = geo["offs_a"][g] + (hf * CH + ci) * K * G
                    av = at[:, a0:a0 + K * G] \
                        .rearrange("p (k g) -> p k g", k=K)
                    return wv, av

                ps_tiles = [psum.tile([128, FF * S], f32, tag="ps",
                                      name=f"ps_{g}_{t}")
                            for t in range(T)]
                # consecutive contexts cycle the NSL column tiles of the
                # PE array so their streams execute concurrently; each
                # context's K accumulating matmuls stay adjacent (the
                # LDWEIGHTS lookahead only tracks one shadow per tile)
                for c in range(CT):
                    wv, av = views(c)
                    t, r2 = divmod(c, CPT)
                    sl, cf = r2 % NSL, r2 // NSL
                    pslice = ps_tiles[t][sl * SLP:sl * SLP + G,
                                         cf * S:cf * S + S]
                    for k in range(K):
                        nc.tensor.matmul(
                            pslice,
                            lhsT=av[:, k, :],
                            rhs=wv[:, k, :],
                            start=(k == 0), stop=(k == K - 1),
                            tile_position=(0, sl * SLP))
                for t in range(T):
                    # one full-width drain per bank; engines alternate per
                    # group, but the tail group always uses the otherwise
                    # idle vector engine (scalar is busy issuing flushes)
                    eng = (nc.vector.tensor_copy
                           if (g % 2 == 0 or g == NG - 1)
                           else nc.scalar.copy)
                    eng(os_t[:, (g * T + t) * FF * S:
                             (g * T + t + 1) * FF * S],
                        ps_tiles[t][:, :])

            # consolidated output flushes: one full-partition DMA per
            # split (garbage rows are cheaper than issue serialization)
            for si, (a, b) in enumerate(geo["splits"]):
                W = (b - a) * OW
                dst = out[128 * a * OW: 128 * b * OW]
                ring = nc.sync if si % 2 == 0 else nc.scalar
                ring.dma_start(
                    dst.rearrange("(p w) -> p w", p=128),
                    os_t[:, a * OW:b * OW])

    nc.compile()
    return nc


def kernel(x, u, WA, WB, adj_xx, adj_xu, context, _trace=False):
    B, S = x.shape
    _, A = u.shape
    C = WA.shape[0]
    assert C % N_CORES == 0
    CP = C // N_CORES
    assert CP % CT == 0
    NG = CP // CT

    # ---- host-side shard: count-sorted contexts, dealt round-robin ----
    context = np.asarray(context)
    cnt = np.bincount(context, minlength=C)
    perm = np.argsort(-cnt, kind="stable")          # contexts by count desc
    # context at global rank r -> core r%8, position r//8; group = pos//CT.
    # All cores share one program, so G_g is set by the chunk's global max
    # count = count at rank g*CT*N_CORES.
    Gs = []
    for g in range(NG):
        m = int(cnt[perm[g * CT * N_CORES]])
        Gs.append(max(2, ((m + 1) // 2) * 2))

    geo = _geometry(S, A, Gs)
    HS, K, CH, WF = geo["HS"], geo["K"], geo["CH"], geo["WF"]
    FF, SLP, NSL, CPT, T, OW = (geo["FF"], geo["SLP"], geo["NSL"],
                                geo["CPT"], geo["T"], geo["OW"])

    order = np.argsort(context, kind="stable")
    starts = np.zeros(C + 1, np.int64)
    starts[1:] = np.cumsum(cnt)

    def group_rows(ctx_ids, G):
        """gidx [len,G] sample indices (clamped) + valid mask."""
        j = np.arange(G)
        cc = cnt[ctx_ids][:, None]
        valid = j[None, :] < cc
        pos = starts[ctx_ids][:, None] + np.minimum(j[None, :],
                                                    np.maximum(cc - 1, 0))
        return order[pos], valid

    inv = np.float32(1.0 / WSCALE)
    x = np.asarray(x, np.float32) * inv
    u = np.asarray(u, np.float32) * inv

    # pre-mask the weight banks, scale into e3m4 range, quantize on host
    Am = (np.asarray(WA, np.float32) * np.float32(WSCALE)
          * np.asarray(adj_xx, np.float32)).astype(FP8)    # [C, S, S]
    Bm = (np.asarray(WB, np.float32) * np.float32(WSCALE)
          * np.asarray(adj_xu, np.float32)).astype(FP8)    # [C, A, S]

    in_maps = []
    scatter = []   # per core: list of (ctx_ids, gidx, valid) per group
    for k in range(N_CORES):
        wblob = np.empty((NG, 128, 2, CH, K, S), FP8)
        ablob = np.zeros((128, geo["AL"]), BF16)
        sc = []
        for g in range(NG):
            G = Gs[g]
            ctx_ids = perm[(g * CT + np.arange(CT)) * N_CORES + k]
            gidx, valid = group_rows(ctx_ids, G)           # [CT, G]
            sc.append((ctx_ids, gidx, valid))
            XpT = x[gidx].transpose(0, 2, 1).astype(BF16)  # [CT, S, G]
            UpT = u[gidx].transpose(0, 2, 1).astype(BF16)  # [CT, A, G]
            wb = wblob[g].transpose(1, 2, 0, 3, 4)         # [2,CH,128,K,S]
            wb[..., 0, :] = Bm[ctx_ids].reshape(2, CH, 128, S)
            wb[..., 1:, :] = Am[ctx_ids].reshape(2, CH, HS, 128, S) \
                .transpose(0, 1, 3, 2, 4)
            A3 = ablob[:, geo["offs_a"][g]:
                       geo["offs_a"][g] + 2 * CH * K * G] \
                .reshape(128, 2, CH, K, G).transpose(1, 2, 3, 0, 4)
            A3[:, :, 0] = UpT.reshape(2, CH, 128, G)
            A3[:, :, 1:] = XpT.reshape(2, CH, HS, 128, G)
        in_maps.append({"wts": wblob.reshape(NG, 128, 2 * WF),
                        "acts": ablob})
        scatter.append(sc)

    if _trace:
        _install_profile_shim()
    nc = _build_program(S, A, Gs)
    res = run_bass_kernel_spmd(nc, in_maps, core_ids=list(range(N_CORES)),
                               trace=_trace)

    # unscatter: each split flush writes its own [128, (b-a)*OW] block
    out_full = np.zeros((B, S), np.float32)
    for k, r in enumerate(res.results):
        v = np.asarray(r["out"]).astype(np.float32)
        for a, b in geo["splits"]:
            blk = v[128 * a * OW: 128 * b * OW] \
                .reshape(128, b - a, T, FF, S)
            for g in range(a, b):
                ctx_ids, gidx, valid = scatter[k][g]
                for c in range(CT):
                    t, r2 = divmod(c, CPT)
                    sl, cf = r2 % NSL, r2 // NSL
                    rows = blk[sl * SLP:sl * SLP + Gs[g],
                               g - a, t, cf, :]              # [G, S]
                    m = valid[c]
                    out_full[gidx[c][m]] = rows[m]

    if _trace:
        return out_full, res

    return out_full
